# revision 8
# baseline (speedup 1.0000x reference)
"""DecoderRNN Trainium2 kernel.

Strategy: 4-way data parallel over batch x 2-way tensor parallel over the
fc vocab dim (8 cores, no collectives).  Each core runs the full LSTM for
its 32-row batch shard and computes logits for its 5000-entry vocab slice.

All GEMMs run with the contraction dim on partitions; the host feeds
pre-transposed weights so only small activation transposes happen on-device.
Matmuls use float32r (full-rate fp32 mode, moving dim >= 256).

Device layout notes (per core):
  - tokens are flattened t-major: flat row = t*32 + b,  t in [0,24), b in [0,32)
  - gx (input-side gate preactivations) are computed as six 128-row M-tiles;
    the time-invariant comb contribution + biases is computed once and
    replicated to 128 partitions, then folded into gx.
  - scan step t reads its gx rows at partition base 32*(t%4) (32-aligned,
    which the engines require).
  - h_t is transposed with DVE 32x32 block transposes + strided copies into
    hsT (H on partitions), which feeds both the next step's W_hh matmul and
    the FC lhsT.  (TensorE transposes would contend with the interleaved FC
    matmuls.)
  - FC weight chunks for most of the vocab stay resident in SBUF and their
    matmul units are emitted inside the scan loop (gated on completed steps)
    so the PE fills the activation-chain gaps and HAM stays warm.
  - SBUF pools are phase-scoped (stack discipline) to stay under the
    192KB/partition budget.
"""

import numpy as np

import concourse.bass as bass
import concourse.mybir as mybir
import concourse.tile as tile
from concourse import bacc
from concourse.bass_utils import run_bass_kernel_spmd
from concourse.masks import make_identity

F32 = mybir.dt.float32
F32R = mybir.dt.float32r
I32 = mybir.dt.int32
AF = mybir.ActivationFunctionType
OP = mybir.AluOpType

# Problem shapes (hardcoded per contest rules).
B, IN, E, H, V, T, NOBJ = 128, 2048, 512, 512, 10000, 25, 10
NCORES = 8
GB = 4                # batch shards
GV = 2                # vocab shards
BL = B // GB          # 32 rows per core
TS = T - 1            # 24 LSTM steps
VL = V // GV          # 5000 vocab entries per core
G4 = 4 * H            # 2048 gate width
NTOK = BL * TS        # 768 tokens per core
NMT = NTOK // 128     # 6 token M-tiles
FCC = 500             # fc N-chunk (fits one PSUM bank)
NFC = VL // FCC       # 10 fc chunks
KH = H // 128         # 4 K-chunks of hidden
KE = E // 128         # 4 K-chunks of embedding
KC = (2 * E) // 128   # 8 K-chunks of comb
KIN = IN // 128       # 16 K-chunks of global features


def _bcast(handle, p):
    """DMA access pattern replicating a 1-D DRAM tensor across p partitions."""
    ap = handle[:]
    return bass.AP(tensor=ap.tensor, offset=ap.offset, ap=[[0, p]] + list(ap.ap))


def _emit_t32(nc, pool, src, dst_fn):
    """Transpose a (32, 512) f32 tile into an H-on-partitions destination.

    DVE 32x32 block-transpose, then 4 strided cast-copies that reposition the
    blocks.  dst_fn(v) must yield a (32, KH, 32) AP at partition base 32*v
    whose [q, k, b] element receives src[b, 128*k + 32*v + q].
    """
    tr = pool.tile([BL, H], F32, tag="htr")
    nc.vector.transpose(out=tr[:], in_=src[:])
    tr4 = tr[:].rearrange("p (k v b) -> p k v b", v=4, b=32)
    for v in range(4):
        nc.vector.tensor_copy(out=dst_fn(v), in_=tr4[:, :, v, :])


def _build(add_fc_bias: bool, add_gate_bias: bool) -> bass.Bass:
    nc = bacc.Bacc(None, target_bir_lowering=False)

    gT_d = nc.dram_tensor("gT", [IN, BL], F32R, kind="ExternalInput")
    objT_d = nc.dram_tensor("objT", [E, BL, NOBJ], F32, kind="ExternalInput")
    capt_d = nc.dram_tensor("capt", [NTOK, 1], I32, kind="ExternalInput")
    emb_d = nc.dram_tensor("emb_table", [V, E], F32, kind="ExternalInput")
    projWT_d = nc.dram_tensor("proj_WT", [IN, E], F32R, kind="ExternalInput")
    projb_d = nc.dram_tensor("proj_b", [E], F32, kind="ExternalInput")
    ihWT_d = nc.dram_tensor("init_hWT", [2 * E, H], F32R, kind="ExternalInput")
    ihb_d = nc.dram_tensor("init_h_b", [H], F32, kind="ExternalInput")
    icWT_d = nc.dram_tensor("init_cWT", [2 * E, H], F32R, kind="ExternalInput")
    icb_d = nc.dram_tensor("init_c_b", [H], F32, kind="ExternalInput")
    wihT_d = nc.dram_tensor("w_ihT", [3 * E, G4], F32R, kind="ExternalInput")
    whhT_d = nc.dram_tensor("w_hhT", [H, G4], F32R, kind="ExternalInput")
    biasg_d = nc.dram_tensor("bias_g", [G4], F32, kind="ExternalInput")
    fcWT_d = nc.dram_tensor("fc_WT", [H, VL], F32R, kind="ExternalInput")
    fcb_d = nc.dram_tensor("fc_b", [VL], F32, kind="ExternalInput")
    out_d = nc.dram_tensor("out", [NTOK, VL], F32, kind="ExternalOutput")

    # When fc bias is active, hold fewer resident fc chunks to fit SBUF.
    n_resident = 6 if add_fc_bias else 8

    with tile.TileContext(nc) as tc:
        with (
            tc.tile_pool(name="persist", bufs=1) as P1,
            tc.tile_pool(name="psq", bufs=1, space="PSUM") as PQ,
            tc.tile_pool(name="psb", bufs=2, space="PSUM") as PB,
            tc.tile_pool(name="pst", bufs=2, space="PSUM") as PT,
        ):
            # Persistent SBUF tiles (~83KB/partition).
            ident = P1.tile([128, 128], F32, tag="ident")
            make_identity(nc, ident)
            identR = P1.tile([128, 128], F32R, tag="identR")
            nc.vector.tensor_copy(out=identR[:], in_=ident[:])
            cst = P1.tile([BL, H], F32, tag="cst")
            h0T = P1.tile([128, KH, BL], F32R, tag="h0T")
            whh = P1.tile([128, KH, G4], F32R, tag="whh")
            gxt = [
                P1.tile([128, G4], F32R, name=f"gx{m}", tag=f"gx{m}")
                for m in range(NMT)
            ]
            # Big persistent weight loads first: they overlap all of phase A.
            nc.sync.dma_start(
                out=whh[:], in_=whhT_d[:].rearrange("(k p) g -> p k g", p=128)
            )

            with tc.tile_pool(name="phAB", bufs=1) as PAB:
                gcombrep = PAB.tile([128, G4], F32, tag="gcombrep")
                wesb = PAB.tile([128, KE, G4], F32R, tag="wesb")
                nc.sync.dma_start(
                    out=wesb[:],
                    in_=wihT_d[0:E, :].rearrange("(k p) g -> p k g", p=128),
                )
                # Embedding gathers kick off immediately (SWDGE queues are
                # idle during the weight loads).
                erows = []
                for m in range(NMT):
                    idxt = PAB.tile([128, 1], I32, name=f"idx{m}", tag=f"idx{m}")
                    nc.sync.dma_start(
                        out=idxt[:], in_=capt_d[128 * m : 128 * (m + 1), :]
                    )
                    erow = PAB.tile([128, E], F32, name=f"erow{m}", tag=f"erow{m}")
                    nc.gpsimd.indirect_dma_start(
                        out=erow[:],
                        out_offset=None,
                        in_=emb_d[:],
                        in_offset=bass.IndirectOffsetOnAxis(ap=idxt[:, :1], axis=0),
                    )
                    erows.append(erow)

                # ---- phase A: comb, h0/c0, gcomb ----
                with (
                    tc.tile_pool(name="phA", bufs=1) as PA,
                    tc.tile_pool(name="wstream", bufs=3) as PW,
                ):
                    projb = PA.tile([BL, E], F32, tag="projb")
                    nc.sync.dma_start(out=projb[:], in_=_bcast(projb_d, BL))
                    ihb = PA.tile([BL, H], F32, tag="ihb")
                    nc.sync.dma_start(out=ihb[:], in_=_bcast(ihb_d, BL))
                    icb = PA.tile([BL, H], F32, tag="icb")
                    nc.sync.dma_start(out=icb[:], in_=_bcast(icb_d, BL))
                    if add_gate_bias:
                        biasg = PA.tile([BL, G4], F32, tag="biasg")
                        nc.sync.dma_start(out=biasg[:], in_=_bcast(biasg_d, BL))

                    gt = PA.tile([128, KIN, BL], F32R, tag="gt")
                    nc.sync.dma_start(
                        out=gt[:], in_=gT_d[:].rearrange("(k p) b -> p k b", p=128)
                    )

                    # feats = global @ proj_W.T + proj_b     (BL, E)
                    ps_proj = PB.tile([BL, E], F32, tag="mm512")
                    for k in range(KIN):
                        pw = PW.tile([128, E], F32R, tag="pw")
                        nc.sync.dma_start(
                            out=pw[:], in_=projWT_d[128 * k : 128 * (k + 1), :]
                        )
                        nc.tensor.matmul(
                            ps_proj[:],
                            gt[:, k, :],
                            pw[:],
                            start=(k == 0),
                            stop=(k == KIN - 1),
                        )
                    feats = PA.tile([BL, E], F32, tag="feats")
                    nc.vector.tensor_tensor(
                        out=feats[:], in0=ps_proj[:], in1=projb[:], op=OP.add
                    )

                    # combT: (128, 8, BL): chunks 0-3 feats.T, 4-7 obj_mean.T
                    combT = PA.tile([128, KC, BL], F32R, tag="combT")
                    _emit_t32(nc, PA, feats, lambda v: combT[32 * v : 32 * (v + 1), 0:4, :])

                    objsb = PA.tile([128, KE, BL, NOBJ], F32, tag="objsb")
                    nc.sync.dma_start(
                        out=objsb[:],
                        in_=objT_d[:].rearrange("(k p) b n -> p k b n", p=128),
                    )
                    objtmp = PA.tile([128, BL], F32, tag="objtmp")
                    for k in range(KE):
                        nc.vector.reduce_sum(
                            out=objtmp[:],
                            in_=objsb[:, k, :, :],
                            axis=mybir.AxisListType.X,
                        )
                        nc.scalar.mul(combT[:, 4 + k, :], objtmp[:], 1.0 / NOBJ)

                    # h0 / c0 = comb @ init_{h,c}_W.T + b
                    for which, (wt_d, bias_t) in enumerate(
                        [(ihWT_d, ihb), (icWT_d, icb)]
                    ):
                        psb_ = PB.tile([BL, H], F32, tag="mm512")
                        for k in range(KC):
                            w = PW.tile([128, H], F32R, tag="pw")
                            nc.sync.dma_start(
                                out=w[:], in_=wt_d[128 * k : 128 * (k + 1), :]
                            )
                            nc.tensor.matmul(
                                psb_[:],
                                combT[:, k, :],
                                w[:],
                                start=(k == 0),
                                stop=(k == KC - 1),
                            )
                        if which == 0:
                            h0sb = PA.tile([BL, H], F32, tag="h0sb")
                            nc.vector.tensor_tensor(
                                out=h0sb[:], in0=psb_[:], in1=bias_t[:], op=OP.add
                            )
                            _emit_t32(
                                nc, PA, h0sb, lambda v: h0T[32 * v : 32 * (v + 1), :, :]
                            )
                        else:
                            nc.vector.tensor_tensor(
                                out=cst[:], in0=psb_[:], in1=bias_t[:], op=OP.add
                            )

                    # gcomb = comb @ Wc.T (+ gate bias), replicated 4x
                    psq = PQ.tile([128, G4], F32, tag="quad")
                    for k in range(KC):
                        wc = PW.tile([128, G4], F32R, tag="wc")
                        nc.sync.dma_start(
                            out=wc[:], in_=wihT_d[E + 128 * k : E + 128 * (k + 1), :]
                        )
                        for n in range(4):
                            nc.tensor.matmul(
                                psq[:BL, 512 * n : 512 * (n + 1)],
                                combT[:, k, :],
                                wc[:, 512 * n : 512 * (n + 1)],
                                start=(k == 0),
                                stop=(k == KC - 1),
                            )
                    gcomb = PA.tile([BL, G4], F32, tag="gcombsb")
                    if add_gate_bias:
                        nc.vector.tensor_tensor(
                            out=gcomb[:], in0=psq[:BL, :], in1=biasg[:], op=OP.add
                        )
                    else:
                        nc.vector.tensor_copy(out=gcomb[:], in_=psq[:BL, :])
                    for j in range(4):
                        nc.vector.tensor_copy(
                            out=gcombrep[32 * j : 32 * (j + 1), :], in_=gcomb[:]
                        )

                # ---- phase B: embedding transposes (PE) + gx M-tiles ----
                for m in range(NMT):
                    embT = PAB.tile(
                        [128, KE, 128], F32R, name=f"embT{m}", tag="embT", bufs=2
                    )
                    for q in range(KE):
                        tp = PT.tile([128, 128], F32, tag="tr")
                        nc.tensor.transpose(
                            out=tp[:],
                            in_=erows[m][:, 128 * q : 128 * (q + 1)],
                            identity=ident[:],
                        )
                        nc.vector.tensor_copy(out=embT[:, q, :], in_=tp[:])
                    psg = PQ.tile([128, G4], F32, tag="quad")
                    for n in range(4):
                        for k in range(KE):
                            nc.tensor.matmul(
                                psg[:, 512 * n : 512 * (n + 1)],
                                embT[:, k, :],
                                wesb[:, k, 512 * n : 512 * (n + 1)],
                                start=(k == 0),
                                stop=(k == KE - 1),
                            )
                    for n in range(4):
                        nc.vector.tensor_tensor(
                            out=gxt[m][:, 512 * n : 512 * (n + 1)],
                            in0=psg[:, 512 * n : 512 * (n + 1)],
                            in1=gcombrep[:, 512 * n : 512 * (n + 1)],
                            op=OP.add,
                        )

            # ---- phase C: LSTM scan with interleaved FC ----
            with (
                tc.tile_pool(name="phC", bufs=1) as PC,
                tc.tile_pool(name="step", bufs=2) as PS,
                tc.tile_pool(name="fcw", bufs=n_resident) as PF,
                tc.tile_pool(name="fcout", bufs=3) as PO,
            ):
                hsT = PC.tile([128, KH, NTOK], F32R, tag="hsT")
                if add_fc_bias:
                    fcb = PC.tile([128, VL], F32, tag="fcb")
                    nc.sync.dma_start(out=fcb[:], in_=_bcast(fcb_d, 128))
                fcw_view = fcWT_d[:].rearrange("(k p) v -> p k v", p=128)
                fw_tiles = {}

                def load_chunk(c):
                    fw = PF.tile([128, KH, FCC], F32R, name=f"fcw{c}", tag="fcw")
                    nc.sync.dma_start(
                        out=fw[:], in_=fcw_view[:, :, FCC * c : FCC * (c + 1)]
                    )
                    fw_tiles[c] = fw

                def emit_fc_unit(m, c):
                    psf = PB.tile([128, FCC], F32, tag="mm512")
                    for k in range(KH):
                        nc.tensor.matmul(
                            psf[:],
                            hsT[:, k, 128 * m : 128 * (m + 1)],
                            fw_tiles[c][:, k, :],
                            start=(k == 0),
                            stop=(k == KH - 1),
                        )
                    ot = PO.tile([128, FCC], F32, tag="fcout")
                    if add_fc_bias:
                        nc.vector.tensor_tensor(
                            out=ot[:],
                            in0=psf[:],
                            in1=fcb[:, FCC * c : FCC * (c + 1)],
                            op=OP.add,
                        )
                    else:
                        nc.vector.tensor_copy(out=ot[:], in_=psf[:])
                    nc.sync.dma_start(
                        out=out_d[128 * m : 128 * (m + 1), FCC * c : FCC * (c + 1)],
                        in_=ot[:],
                    )

                for c in range(n_resident):
                    load_chunk(c)

                pending = []
                for t in range(TS):
                    m_t, j = divmod(t, 4)
                    base = 32 * j
                    psg = PQ.tile([BL, G4], F32, tag="quad")
                    # gate banks are host-permuted to (g, i, f, o) so the
                    # c-path activations pipeline under the matmul burst
                    for n in range(4):
                        for k in range(KH):
                            lhs = (
                                h0T[:, k, :]
                                if t == 0
                                else hsT[:, k, 32 * (t - 1) : 32 * t]
                            )
                            nc.tensor.matmul(
                                psg[:, 512 * n : 512 * (n + 1)],
                                lhs,
                                whh[:, k, 512 * n : 512 * (n + 1)],
                                start=(k == 0),
                                stop=False,
                            )
                        # fold gx in with a 32x32 identity matmul instead of a
                        # serial DVE add (frees the DVE/ACT critical path)
                        nc.tensor.matmul(
                            psg[:, 512 * n : 512 * (n + 1)],
                            identR[base : base + BL, base : base + BL],
                            gxt[m_t][base : base + BL, 512 * n : 512 * (n + 1)],
                            start=False,
                            stop=True,
                            tile_position=(base, 0),
                        )
                    gta = PS.tile([BL, H], F32, tag="gta")
                    nc.scalar.activation(gta[:], psg[:, 0:512], AF.Tanh)
                    ig = PS.tile([BL, H], F32, tag="ig")
                    nc.scalar.activation(ig[:], psg[:, 512:1024], AF.Sigmoid)
                    fg = PS.tile([BL, H], F32, tag="fg")
                    nc.scalar.activation(fg[:], psg[:, 1024:1536], AF.Sigmoid)
                    og = PS.tile([BL, H], F32, tag="og")
                    nc.scalar.activation(og[:], psg[:, 1536:2048], AF.Sigmoid)
                    # c = f*c + i*g (in-place; overlaps the tail of the burst)
                    nc.vector.tensor_tensor(out=ig[:], in0=ig[:], in1=gta[:], op=OP.mult)
                    nc.vector.tensor_tensor(out=fg[:], in0=fg[:], in1=cst[:], op=OP.mult)
                    nc.vector.tensor_tensor(out=cst[:], in0=fg[:], in1=ig[:], op=OP.add)
                    # h = o*tanh(c), transposed into hsT, pipelined by quarters
                    # so next step's k=0 matmuls start after the first quarter
                    tr = PS.tile([BL, H], F32, tag="htr")
                    for q in range(KH):
                        sl = slice(128 * q, 128 * (q + 1))
                        nc.scalar.activation(gta[:, sl], cst[:, sl], AF.Tanh)
                        nc.vector.tensor_tensor(
                            out=og[:, sl], in0=og[:, sl], in1=gta[:, sl], op=OP.mult
                        )
                        nc.vector.transpose(out=tr[:, sl], in_=og[:, sl])
                        for v in range(4):
                            nc.vector.tensor_copy(
                                out=hsT[32 * v : 32 * (v + 1), q, 32 * t : 32 * (t + 1)],
                                in_=tr[:, 128 * q + 32 * v : 128 * q + 32 * (v + 1)],
                            )
                    # interleave ready FC units into the activation-chain gap
                    if j == 3 and m_t < 5:
                        pending.extend((m_t, c) for c in range(n_resident))
                    for _ in range(2):
                        if pending:
                            emit_fc_unit(*pending.pop(0))

                # ---- FC tail: last M-tile + non-resident chunks ----
                while pending:
                    emit_fc_unit(*pending.pop(0))
                for c in range(n_resident):
                    emit_fc_unit(5, c)
                for c in range(n_resident, NFC):
                    load_chunk(c)
                    for m in range(NMT):
                        emit_fc_unit(m, c)
    nc.compile()
    return nc


_NC_CACHE: dict = {}


def _get_nc(add_fc_bias: bool, add_gate_bias: bool) -> bass.Bass:
    key = (add_fc_bias, add_gate_bias)
    if key not in _NC_CACHE:
        _NC_CACHE[key] = _build(*key)
    return _NC_CACHE[key]


def _make_in_maps(inputs):
    gf = np.ascontiguousarray(np.asarray(inputs["global_features"], dtype=np.float32))
    of = np.ascontiguousarray(np.asarray(inputs["object_features"], dtype=np.float32))
    capt = np.asarray(inputs["captions"]).astype(np.int32)
    emb = np.ascontiguousarray(np.asarray(inputs["emb_table"], dtype=np.float32))
    projWT = np.ascontiguousarray(np.asarray(inputs["proj_W"], dtype=np.float32).T)
    ihWT = np.ascontiguousarray(np.asarray(inputs["init_h_W"], dtype=np.float32).T)
    icWT = np.ascontiguousarray(np.asarray(inputs["init_c_W"], dtype=np.float32).T)
    def permute_gates(a):
        # reference gate order (i, f, g, o) -> device order (g, i, f, o)
        return np.ascontiguousarray(
            np.concatenate(
                [a[..., 1024:1536], a[..., 0:512], a[..., 512:1024], a[..., 1536:2048]],
                axis=-1,
            )
        )

    wihT = permute_gates(np.asarray(inputs["W_ih"], dtype=np.float32).T)
    whhT = permute_gates(np.asarray(inputs["W_hh"], dtype=np.float32).T)
    fcWT = np.ascontiguousarray(np.asarray(inputs["fc_W"], dtype=np.float32).T)
    projb = np.asarray(inputs["proj_b"], dtype=np.float32)
    ihb = np.asarray(inputs["init_h_b"], dtype=np.float32)
    icb = np.asarray(inputs["init_c_b"], dtype=np.float32)
    biasg = permute_gates(
        np.asarray(inputs["b_ih"], dtype=np.float32)
        + np.asarray(inputs["b_hh"], dtype=np.float32)
    )
    fcb = np.asarray(inputs["fc_b"], dtype=np.float32)
    add_fc_bias = bool(np.any(fcb))
    add_gate_bias = bool(np.any(biasg))

    in_maps = []
    for c in range(NCORES):
        b4, vh = c % GB, c // GB
        rows = slice(BL * b4, BL * (b4 + 1))
        vsl = slice(VL * vh, VL * (vh + 1))
        in_maps.append(
            {
                "gT": np.ascontiguousarray(gf[rows].T),
                "objT": np.ascontiguousarray(of[rows].transpose(2, 0, 1)),
                "capt": np.ascontiguousarray(
                    capt[rows, : T - 1].T.reshape(NTOK, 1)
                ),
                "emb_table": emb,
                "proj_WT": projWT,
                "proj_b": projb,
                "init_hWT": ihWT,
                "init_h_b": ihb,
                "init_cWT": icWT,
                "init_c_b": icb,
                "w_ihT": wihT,
                "w_hhT": whhT,
                "bias_g": biasg,
                "fc_WT": np.ascontiguousarray(fcWT[:, vsl]),
                "fc_b": np.ascontiguousarray(fcb[vsl]),
            }
        )
    return in_maps, add_fc_bias, add_gate_bias


def _unshard(results):
    out = np.empty((B, TS, V), dtype=np.float32)
    for c in range(NCORES):
        b4, vh = c % GB, c // GB
        rows = slice(BL * b4, BL * (b4 + 1))
        vsl = slice(VL * vh, VL * (vh + 1))
        o = np.asarray(results[c]["out"]).reshape(TS, BL, VL).transpose(1, 0, 2)
        out[rows, :, vsl] = o
    return out


def _run(inputs, trace=False):
    in_maps, add_fc_bias, add_gate_bias = _make_in_maps(inputs)
    nc = _get_nc(add_fc_bias, add_gate_bias)
    res = run_bass_kernel_spmd(nc, in_maps, list(range(NCORES)), trace=trace)
    return _unshard(res.results), res.exec_time_ns


def kernel(**inputs) -> np.ndarray:
    out, _ = _run(inputs, trace=False)
    return out


# revision 9
# speedup vs baseline: 1.4049x; 1.4049x over previous
"""DecoderRNN Trainium2 kernel.

Strategy: 4-way data parallel over batch x 2-way tensor parallel over the
fc vocab dim (8 cores, no collectives).  Each core runs the full LSTM for
its 32-row batch shard and computes logits for its 5000-entry vocab slice.

All GEMMs run with the contraction dim on partitions; the host feeds
pre-transposed weights so only small activation transposes happen on-device.
Matmuls use float32r (full-rate fp32 mode, moving dim >= 256).

Device layout notes (per core):
  - tokens are flattened t-major: flat row = t*32 + b,  t in [0,24), b in [0,32)
  - gx (input-side gate preactivations) are computed as six 128-row M-tiles;
    the time-invariant comb contribution + biases is computed once and
    replicated to 128 partitions, then folded into gx.
  - scan step t reads its gx rows at partition base 32*(t%4) (32-aligned,
    which the engines require).
  - h_t is transposed with DVE 32x32 block transposes + strided copies into
    hsT (H on partitions), which feeds both the next step's W_hh matmul and
    the FC lhsT.  (TensorE transposes would contend with the interleaved FC
    matmuls.)
  - FC weight chunks for most of the vocab stay resident in SBUF and their
    matmul units are emitted inside the scan loop (gated on completed steps)
    so the PE fills the activation-chain gaps and HAM stays warm.
  - SBUF pools are phase-scoped (stack discipline) to stay under the
    192KB/partition budget.
"""

import numpy as np

import concourse.bass as bass
import concourse.mybir as mybir
import concourse.tile as tile
from concourse import bacc
from concourse.bass_utils import run_bass_kernel_spmd
from concourse.masks import make_identity

F32 = mybir.dt.float32
F32R = mybir.dt.float32r
I32 = mybir.dt.int32
AF = mybir.ActivationFunctionType
OP = mybir.AluOpType

# Problem shapes (hardcoded per contest rules).
B, IN, E, H, V, T, NOBJ = 128, 2048, 512, 512, 10000, 25, 10
NCORES = 8
GB = 4                # batch shards
GV = 2                # vocab shards
BL = B // GB          # 32 rows per core
TS = T - 1            # 24 LSTM steps
VL = V // GV          # 5000 vocab entries per core
G4 = 4 * H            # 2048 gate width
NTOK = BL * TS        # 768 tokens per core
NMT = NTOK // 128     # 6 token M-tiles
FCC = 500             # fc N-chunk (fits one PSUM bank)
NFC = VL // FCC       # 10 fc chunks
KH = H // 128         # 4 K-chunks of hidden
KE = E // 128         # 4 K-chunks of embedding
KC = (2 * E) // 128   # 8 K-chunks of comb
KIN = IN // 128       # 16 K-chunks of global features


def _bcast(handle, p):
    """DMA access pattern replicating a 1-D DRAM tensor across p partitions."""
    ap = handle[:]
    return bass.AP(tensor=ap.tensor, offset=ap.offset, ap=[[0, p]] + list(ap.ap))


def _emit_t32(nc, pool, src, dst_fn):
    """Transpose a (32, 512) f32 tile into an H-on-partitions destination.

    DVE 32x32 block-transpose, then 4 strided cast-copies that reposition the
    blocks.  dst_fn(v) must yield a (32, KH, 32) AP at partition base 32*v
    whose [q, k, b] element receives src[b, 128*k + 32*v + q].
    """
    tr = pool.tile([BL, H], F32, tag="htr")
    nc.vector.transpose(out=tr[:], in_=src[:])
    tr4 = tr[:].rearrange("p (k v b) -> p k v b", v=4, b=32)
    for v in range(4):
        nc.vector.tensor_copy(out=dst_fn(v), in_=tr4[:, :, v, :])


def _build(add_fc_bias: bool, add_gate_bias: bool) -> bass.Bass:
    nc = bacc.Bacc(None, target_bir_lowering=False)

    gT_d = nc.dram_tensor("gT", [IN, BL], F32R, kind="ExternalInput")
    objT_d = nc.dram_tensor("objT", [E, BL, NOBJ], F32, kind="ExternalInput")
    capt_d = nc.dram_tensor("capt", [NTOK, 1], I32, kind="ExternalInput")
    emb_d = nc.dram_tensor("emb_table", [V, E], F32, kind="ExternalInput")
    projWT_d = nc.dram_tensor("proj_WT", [IN, E], F32R, kind="ExternalInput")
    projb_d = nc.dram_tensor("proj_b", [E], F32, kind="ExternalInput")
    ihWT_d = nc.dram_tensor("init_hWT", [2 * E, H], F32R, kind="ExternalInput")
    ihb_d = nc.dram_tensor("init_h_b", [H], F32, kind="ExternalInput")
    icWT_d = nc.dram_tensor("init_cWT", [2 * E, H], F32R, kind="ExternalInput")
    icb_d = nc.dram_tensor("init_c_b", [H], F32, kind="ExternalInput")
    wihT_d = nc.dram_tensor("w_ihT", [3 * E, G4], F32R, kind="ExternalInput")
    whhT_d = nc.dram_tensor("w_hhT", [H, G4], F32R, kind="ExternalInput")
    biasg_d = nc.dram_tensor("bias_g", [G4], F32, kind="ExternalInput")
    fcWT_d = nc.dram_tensor("fc_WT", [H, VL], F32R, kind="ExternalInput")
    fcb_d = nc.dram_tensor("fc_b", [VL], F32, kind="ExternalInput")
    out_d = nc.dram_tensor("out", [NTOK, VL], F32, kind="ExternalOutput")

    # When fc bias is active, hold fewer resident fc chunks to fit SBUF.
    n_resident = 6 if add_fc_bias else 8

    with tile.TileContext(nc) as tc:
        with (
            tc.tile_pool(name="persist", bufs=1) as P1,
            tc.tile_pool(name="psq", bufs=1, space="PSUM") as PQ,
            tc.tile_pool(name="psb", bufs=2, space="PSUM") as PB,
            tc.tile_pool(name="pst", bufs=2, space="PSUM") as PT,
        ):
            # Persistent SBUF tiles (~83KB/partition).
            ident = P1.tile([128, 128], F32, tag="ident")
            make_identity(nc, ident)
            identR = P1.tile([128, 128], F32R, tag="identR")
            nc.vector.tensor_copy(out=identR[:], in_=ident[:])
            cst = P1.tile([BL, H], F32, tag="cst")
            h0T = P1.tile([128, KH, BL], F32R, tag="h0T")
            whh = P1.tile([128, KH, G4], F32R, tag="whh")
            gxt = [
                P1.tile([128, G4], F32R, name=f"gx{m}", tag=f"gx{m}")
                for m in range(NMT)
            ]
            # Big persistent weight loads first: they overlap all of phase A.
            nc.sync.dma_start(
                out=whh[:], in_=whhT_d[:].rearrange("(k p) g -> p k g", p=128)
            )

            with tc.tile_pool(name="phAB", bufs=1) as PAB:
                gcombrep = PAB.tile([128, G4], F32, tag="gcombrep")
                wesb = PAB.tile([128, KE, G4], F32R, tag="wesb")
                nc.sync.dma_start(
                    out=wesb[:],
                    in_=wihT_d[0:E, :].rearrange("(k p) g -> p k g", p=128),
                )
                # Embedding gathers kick off immediately (SWDGE queues are
                # idle during the weight loads).
                erows = []
                for m in range(NMT):
                    idxt = PAB.tile([128, 1], I32, name=f"idx{m}", tag=f"idx{m}")
                    nc.sync.dma_start(
                        out=idxt[:], in_=capt_d[128 * m : 128 * (m + 1), :]
                    )
                    erow = PAB.tile([128, E], F32, name=f"erow{m}", tag=f"erow{m}")
                    nc.gpsimd.indirect_dma_start(
                        out=erow[:],
                        out_offset=None,
                        in_=emb_d[:],
                        in_offset=bass.IndirectOffsetOnAxis(ap=idxt[:, :1], axis=0),
                    )
                    erows.append(erow)

                # ---- phase A: comb, h0/c0, gcomb ----
                with (
                    tc.tile_pool(name="phA", bufs=1) as PA,
                    tc.tile_pool(name="wstream", bufs=3) as PW,
                ):
                    projb = PA.tile([BL, E], F32, tag="projb")
                    nc.sync.dma_start(out=projb[:], in_=_bcast(projb_d, BL))
                    ihb = PA.tile([BL, H], F32, tag="ihb")
                    nc.sync.dma_start(out=ihb[:], in_=_bcast(ihb_d, BL))
                    icb = PA.tile([BL, H], F32, tag="icb")
                    nc.sync.dma_start(out=icb[:], in_=_bcast(icb_d, BL))
                    if add_gate_bias:
                        biasg = PA.tile([BL, G4], F32, tag="biasg")
                        nc.sync.dma_start(out=biasg[:], in_=_bcast(biasg_d, BL))

                    gt = PA.tile([128, KIN, BL], F32R, tag="gt")
                    nc.sync.dma_start(
                        out=gt[:], in_=gT_d[:].rearrange("(k p) b -> p k b", p=128)
                    )

                    # feats = global @ proj_W.T + proj_b     (BL, E)
                    ps_proj = PB.tile([BL, E], F32, tag="mm512")
                    for k in range(KIN):
                        pw = PW.tile([128, E], F32R, tag="pw")
                        nc.sync.dma_start(
                            out=pw[:], in_=projWT_d[128 * k : 128 * (k + 1), :]
                        )
                        nc.tensor.matmul(
                            ps_proj[:],
                            gt[:, k, :],
                            pw[:],
                            start=(k == 0),
                            stop=(k == KIN - 1),
                        )
                    feats = PA.tile([BL, E], F32, tag="feats")
                    nc.vector.tensor_tensor(
                        out=feats[:], in0=ps_proj[:], in1=projb[:], op=OP.add
                    )

                    # combT: (128, 8, BL): chunks 0-3 feats.T, 4-7 obj_mean.T
                    combT = PA.tile([128, KC, BL], F32R, tag="combT")
                    _emit_t32(nc, PA, feats, lambda v: combT[32 * v : 32 * (v + 1), 0:4, :])

                    objsb = PA.tile([128, KE, BL, NOBJ], F32, tag="objsb")
                    nc.sync.dma_start(
                        out=objsb[:],
                        in_=objT_d[:].rearrange("(k p) b n -> p k b n", p=128),
                    )
                    objtmp = PA.tile([128, BL], F32, tag="objtmp")
                    for k in range(KE):
                        nc.vector.reduce_sum(
                            out=objtmp[:],
                            in_=objsb[:, k, :, :],
                            axis=mybir.AxisListType.X,
                        )
                        nc.scalar.mul(combT[:, 4 + k, :], objtmp[:], 1.0 / NOBJ)

                    # h0 / c0 = comb @ init_{h,c}_W.T + b
                    for which, (wt_d, bias_t) in enumerate(
                        [(ihWT_d, ihb), (icWT_d, icb)]
                    ):
                        psb_ = PB.tile([BL, H], F32, tag="mm512")
                        for k in range(KC):
                            w = PW.tile([128, H], F32R, tag="pw")
                            nc.sync.dma_start(
                                out=w[:], in_=wt_d[128 * k : 128 * (k + 1), :]
                            )
                            nc.tensor.matmul(
                                psb_[:],
                                combT[:, k, :],
                                w[:],
                                start=(k == 0),
                                stop=(k == KC - 1),
                            )
                        if which == 0:
                            h0sb = PA.tile([BL, H], F32, tag="h0sb")
                            nc.vector.tensor_tensor(
                                out=h0sb[:], in0=psb_[:], in1=bias_t[:], op=OP.add
                            )
                            _emit_t32(
                                nc, PA, h0sb, lambda v: h0T[32 * v : 32 * (v + 1), :, :]
                            )
                        else:
                            nc.vector.tensor_tensor(
                                out=cst[:], in0=psb_[:], in1=bias_t[:], op=OP.add
                            )

                    # gcomb = comb @ Wc.T (+ gate bias), replicated 4x
                    psqb = [
                        PQ.tile([BL, 512], F32, name=f"gcq{n}", tag=f"qb{n}")
                        for n in range(4)
                    ]
                    for k in range(KC):
                        wc = PW.tile([128, G4], F32R, tag="wc")
                        nc.sync.dma_start(
                            out=wc[:], in_=wihT_d[E + 128 * k : E + 128 * (k + 1), :]
                        )
                        for n in range(4):
                            nc.tensor.matmul(
                                psqb[n][:],
                                combT[:, k, :],
                                wc[:, 512 * n : 512 * (n + 1)],
                                start=(k == 0),
                                stop=(k == KC - 1),
                            )
                    gcomb = PA.tile([BL, G4], F32, tag="gcombsb")
                    for n in range(4):
                        if add_gate_bias:
                            nc.vector.tensor_tensor(
                                out=gcomb[:, 512 * n : 512 * (n + 1)],
                                in0=psqb[n][:],
                                in1=biasg[:, 512 * n : 512 * (n + 1)],
                                op=OP.add,
                            )
                        else:
                            nc.vector.tensor_copy(
                                out=gcomb[:, 512 * n : 512 * (n + 1)], in_=psqb[n][:]
                            )
                    for j in range(4):
                        nc.vector.tensor_copy(
                            out=gcombrep[32 * j : 32 * (j + 1), :], in_=gcomb[:]
                        )

                # ---- phase B: embedding transposes (PE) + gx M-tiles ----
                for m in range(NMT):
                    embT = PAB.tile(
                        [128, KE, 128], F32R, name=f"embT{m}", tag="embT", bufs=2
                    )
                    for q in range(KE):
                        tp = PT.tile([128, 128], F32, tag="tr")
                        nc.tensor.transpose(
                            out=tp[:],
                            in_=erows[m][:, 128 * q : 128 * (q + 1)],
                            identity=ident[:],
                        )
                        nc.vector.tensor_copy(out=embT[:, q, :], in_=tp[:])
                    psgb = [
                        PQ.tile([128, 512], F32, name=f"gxq{n}_{m}", tag=f"qb{n}")
                        for n in range(4)
                    ]
                    for n in range(4):
                        for k in range(KE):
                            nc.tensor.matmul(
                                psgb[n][:],
                                embT[:, k, :],
                                wesb[:, k, 512 * n : 512 * (n + 1)],
                                start=(k == 0),
                                stop=(k == KE - 1),
                            )
                    for n in range(4):
                        nc.vector.tensor_tensor(
                            out=gxt[m][:, 512 * n : 512 * (n + 1)],
                            in0=psgb[n][:],
                            in1=gcombrep[:, 512 * n : 512 * (n + 1)],
                            op=OP.add,
                        )

            # ---- phase C: LSTM scan with interleaved FC ----
            with (
                tc.tile_pool(name="phC", bufs=1) as PC,
                tc.tile_pool(name="step", bufs=2) as PS,
                tc.tile_pool(name="fcw", bufs=n_resident) as PF,
                tc.tile_pool(name="fcout", bufs=3) as PO,
            ):
                hsT = PC.tile([128, KH, NTOK], F32R, tag="hsT")
                if add_fc_bias:
                    fcb = PC.tile([128, VL], F32, tag="fcb")
                    nc.sync.dma_start(out=fcb[:], in_=_bcast(fcb_d, 128))
                fcw_view = fcWT_d[:].rearrange("(k p) v -> p k v", p=128)
                fw_tiles = {}

                def load_chunk(c):
                    fw = PF.tile([128, KH, FCC], F32R, name=f"fcw{c}", tag="fcw")
                    nc.sync.dma_start(
                        out=fw[:], in_=fcw_view[:, :, FCC * c : FCC * (c + 1)]
                    )
                    fw_tiles[c] = fw

                def emit_fc_unit(m, c):
                    psf = PB.tile([128, FCC], F32, tag="mm512")
                    for k in range(KH):
                        nc.tensor.matmul(
                            psf[:],
                            hsT[:, k, 128 * m : 128 * (m + 1)],
                            fw_tiles[c][:, k, :],
                            start=(k == 0),
                            stop=(k == KH - 1),
                        )
                    ot = PO.tile([128, FCC], F32, tag="fcout")
                    if add_fc_bias:
                        nc.vector.tensor_tensor(
                            out=ot[:],
                            in0=psf[:],
                            in1=fcb[:, FCC * c : FCC * (c + 1)],
                            op=OP.add,
                        )
                    else:
                        nc.vector.tensor_copy(out=ot[:], in_=psf[:])
                    nc.sync.dma_start(
                        out=out_d[128 * m : 128 * (m + 1), FCC * c : FCC * (c + 1)],
                        in_=ot[:],
                    )

                for c in range(n_resident):
                    load_chunk(c)

                pending = []
                for t in range(TS):
                    m_t, j = divmod(t, 4)
                    base = 32 * j
                    # one PSUM tile per gate bank so per-bank activations
                    # start as soon as that bank's accumulation completes
                    psgb = [
                        PQ.tile([BL, 512], F32, name=f"qb{n}_{t}", tag=f"qb{n}")
                        for n in range(4)
                    ]
                    # gate banks are host-permuted to (g, i, f, o) so the
                    # c-path activations pipeline under the matmul burst
                    for n in range(4):
                        for k in range(KH):
                            lhs = (
                                h0T[:, k, :]
                                if t == 0
                                else hsT[:, k, 32 * (t - 1) : 32 * t]
                            )
                            nc.tensor.matmul(
                                psgb[n][:],
                                lhs,
                                whh[:, k, 512 * n : 512 * (n + 1)],
                                start=(k == 0),
                                stop=False,
                            )
                        # fold gx in with a 32x32 identity matmul instead of a
                        # serial DVE add (frees the DVE/ACT critical path)
                        nc.tensor.matmul(
                            psgb[n][:],
                            identR[base : base + BL, base : base + BL],
                            gxt[m_t][base : base + BL, 512 * n : 512 * (n + 1)],
                            start=False,
                            stop=True,
                            tile_position=(base, 0),
                        )
                    gta = PS.tile([BL, H], F32, tag="gta")
                    nc.scalar.activation(gta[:], psgb[0][:], AF.Tanh)
                    ig = PS.tile([BL, H], F32, tag="ig")
                    nc.scalar.activation(ig[:], psgb[1][:], AF.Sigmoid)
                    fg = PS.tile([BL, H], F32, tag="fg")
                    nc.scalar.activation(fg[:], psgb[2][:], AF.Sigmoid)
                    og = PS.tile([BL, H], F32, tag="og")
                    nc.scalar.activation(og[:], psgb[3][:], AF.Sigmoid)
                    # c = f*c + i*g (in-place; overlaps the tail of the burst)
                    nc.vector.tensor_tensor(out=ig[:], in0=ig[:], in1=gta[:], op=OP.mult)
                    nc.vector.tensor_tensor(out=fg[:], in0=fg[:], in1=cst[:], op=OP.mult)
                    nc.vector.tensor_tensor(out=cst[:], in0=fg[:], in1=ig[:], op=OP.add)
                    # h = o*tanh(c), transposed into hsT, pipelined by quarters
                    # so next step's k=0 matmuls start after the first quarter
                    tr = PS.tile([BL, H], F32, tag="htr")
                    for q in range(KH):
                        sl = slice(128 * q, 128 * (q + 1))
                        nc.scalar.activation(gta[:, sl], cst[:, sl], AF.Tanh)
                        nc.vector.tensor_tensor(
                            out=og[:, sl], in0=og[:, sl], in1=gta[:, sl], op=OP.mult
                        )
                        nc.vector.transpose(out=tr[:, sl], in_=og[:, sl])
                        for v in range(4):
                            nc.vector.tensor_copy(
                                out=hsT[32 * v : 32 * (v + 1), q, 32 * t : 32 * (t + 1)],
                                in_=tr[:, 128 * q + 32 * v : 128 * q + 32 * (v + 1)],
                            )
                    # interleave ready FC units into the activation-chain gap
                    if j == 3 and m_t < 5:
                        pending.extend((m_t, c) for c in range(n_resident))
                    for _ in range(2):
                        if pending:
                            emit_fc_unit(*pending.pop(0))

                # ---- FC tail: last M-tile + non-resident chunks ----
                while pending:
                    emit_fc_unit(*pending.pop(0))
                for c in range(n_resident):
                    emit_fc_unit(5, c)
                for c in range(n_resident, NFC):
                    load_chunk(c)
                    for m in range(NMT):
                        emit_fc_unit(m, c)
    nc.compile()
    return nc


_NC_CACHE: dict = {}


def _get_nc(add_fc_bias: bool, add_gate_bias: bool) -> bass.Bass:
    key = (add_fc_bias, add_gate_bias)
    if key not in _NC_CACHE:
        _NC_CACHE[key] = _build(*key)
    return _NC_CACHE[key]


def _make_in_maps(inputs):
    gf = np.ascontiguousarray(np.asarray(inputs["global_features"], dtype=np.float32))
    of = np.ascontiguousarray(np.asarray(inputs["object_features"], dtype=np.float32))
    capt = np.asarray(inputs["captions"]).astype(np.int32)
    emb = np.ascontiguousarray(np.asarray(inputs["emb_table"], dtype=np.float32))
    projWT = np.ascontiguousarray(np.asarray(inputs["proj_W"], dtype=np.float32).T)
    ihWT = np.ascontiguousarray(np.asarray(inputs["init_h_W"], dtype=np.float32).T)
    icWT = np.ascontiguousarray(np.asarray(inputs["init_c_W"], dtype=np.float32).T)
    def permute_gates(a):
        # reference gate order (i, f, g, o) -> device order (g, i, f, o)
        return np.ascontiguousarray(
            np.concatenate(
                [a[..., 1024:1536], a[..., 0:512], a[..., 512:1024], a[..., 1536:2048]],
                axis=-1,
            )
        )

    wihT = permute_gates(np.asarray(inputs["W_ih"], dtype=np.float32).T)
    whhT = permute_gates(np.asarray(inputs["W_hh"], dtype=np.float32).T)
    fcWT = np.ascontiguousarray(np.asarray(inputs["fc_W"], dtype=np.float32).T)
    projb = np.asarray(inputs["proj_b"], dtype=np.float32)
    ihb = np.asarray(inputs["init_h_b"], dtype=np.float32)
    icb = np.asarray(inputs["init_c_b"], dtype=np.float32)
    biasg = permute_gates(
        np.asarray(inputs["b_ih"], dtype=np.float32)
        + np.asarray(inputs["b_hh"], dtype=np.float32)
    )
    fcb = np.asarray(inputs["fc_b"], dtype=np.float32)
    add_fc_bias = bool(np.any(fcb))
    add_gate_bias = bool(np.any(biasg))

    in_maps = []
    for c in range(NCORES):
        b4, vh = c % GB, c // GB
        rows = slice(BL * b4, BL * (b4 + 1))
        vsl = slice(VL * vh, VL * (vh + 1))
        in_maps.append(
            {
                "gT": np.ascontiguousarray(gf[rows].T),
                "objT": np.ascontiguousarray(of[rows].transpose(2, 0, 1)),
                "capt": np.ascontiguousarray(
                    capt[rows, : T - 1].T.reshape(NTOK, 1)
                ),
                "emb_table": emb,
                "proj_WT": projWT,
                "proj_b": projb,
                "init_hWT": ihWT,
                "init_h_b": ihb,
                "init_cWT": icWT,
                "init_c_b": icb,
                "w_ihT": wihT,
                "w_hhT": whhT,
                "bias_g": biasg,
                "fc_WT": np.ascontiguousarray(fcWT[:, vsl]),
                "fc_b": np.ascontiguousarray(fcb[vsl]),
            }
        )
    return in_maps, add_fc_bias, add_gate_bias


def _unshard(results):
    out = np.empty((B, TS, V), dtype=np.float32)
    for c in range(NCORES):
        b4, vh = c % GB, c // GB
        rows = slice(BL * b4, BL * (b4 + 1))
        vsl = slice(VL * vh, VL * (vh + 1))
        o = np.asarray(results[c]["out"]).reshape(TS, BL, VL).transpose(1, 0, 2)
        out[rows, :, vsl] = o
    return out


def _run(inputs, trace=False):
    in_maps, add_fc_bias, add_gate_bias = _make_in_maps(inputs)
    nc = _get_nc(add_fc_bias, add_gate_bias)
    res = run_bass_kernel_spmd(nc, in_maps, list(range(NCORES)), trace=trace)
    return _unshard(res.results), res.exec_time_ns


def kernel(**inputs) -> np.ndarray:
    out, _ = _run(inputs, trace=False)
    return out
